# revision 26
# baseline (speedup 1.0000x reference)
"""Trainium2 Bass kernel for MemoryEfficientFlashAttention (B=2,S=2048,HID=2048,H=16,HKV=8,D=128,CHUNK=512).

Sharding: 8 cores = 2 batches x 4 head-groups (4 q heads / 2 kv heads per core).
Each core computes q/k/v projections (+RoPE), the chunked flash-attention
recurrence, and a row-sharded partial of the output projection (transposed).
Host sums the 4 partials per batch and adds bo.

Math: the reference's scan step is algebraically
    o_j = (o_{j-1} * e^{m_{j-1}} + Y_j) / (e^{m_{j-1}} + S_j)
with Y_j = exp(sc_j) @ V_j, S_j = rowsum exp(sc_j), m_j = running max.
Unrolled with the trailing o/d divide:
    o = sum_j (RawE_j @ V) * c_j,   RawE_j = exp(sc_j)  (raw, no max subtract;
    scores are O(6) so exp cannot overflow),
    c_j[q] = 1 / (M_n * prod_{l>=j} d_l * d_n^flag),  d_l = (M_{l-1} + S_l)/M_l,
    M_l = running max of exp scores (M_{-1} = 0), flag = processed the
    globally-last kv chunk (reproduces the reference's final o/d divide).

Pass 1 computes scores [q,k] once per chunk, exp's them into a RESIDENT bf16
P tile (plus Act-accumulated raw row sums and DVE row maxes); the (M, S) chain
runs entirely in the exp domain on DVE (mul/max/reciprocal - no Ln/Exp chain
ops).  Pass 2 never recomputes scores: each 128x128 P block is multiplied by
diag(c_t) on the tensor engine (a plain matmul with a diagonal rhs built via
one tensor_scalar per block column), which transposes AND scales in one pass;
the fp32 PSUM result is copied to bf16 SBUF (Act/DVE) and fed to the PV
matmuls accumulating u = sum_j c_j*(E_j^T) @ V directly in PSUM.

Perf structure: bf16 operands for all large matmuls, causal narrowing of
diagonal chunks, single shared 128x128 triangular mask tile, weights resident
in SBUF, v-projection run early on the still-loaded x chunk (no reloads), and
per-(qi,h) pass1->pass2 pipelining so only ~2 P tiles are ever live.
"""

import os
import sys
from contextlib import ExitStack

import numpy as np
import ml_dtypes

sys.path.insert(0, "/opt/trn_rl_repo")
os.environ.setdefault("MYCRO_LOCAL_CACHE", "1")

import concourse.bass as bass  # noqa: E402
import concourse.tile as tile  # noqa: E402
from concourse import bacc, mybir  # noqa: E402
from concourse.bass_utils import run_bass_kernel_spmd  # noqa: E402

# Steer insert_act_table_loads to a table set that holds Exp (and Copy),
# so the kernel loads one activation table total.
import collections  # noqa: E402
import concourse.hw_specs as _hw_specs  # noqa: E402

_gat_orig = _hw_specs.get_activation_tables


def _gat_combined(arch):
    tabs = _gat_orig(arch)
    both = {mybir.ActivationFunctionType.Exp, mybir.ActivationFunctionType.Ln}
    out = collections.OrderedDict()
    for name, s in tabs.items():
        if name == "natural_log_exp_and_others" or not (s & both):
            out[name] = s
        else:
            out[name] = s - both
    return out


bacc.get_activation_tables = _gat_combined

B, S, HID = 2, 2048, 2048
H, HKV, D = 16, 8, 128
CHUNK = 512
THETA = 1000000.0
NEG = -1e9
NCORES = 8
HL = H // (NCORES // B)      # 4 local q heads
KVL = HKV // (NCORES // B)   # 2 local kv heads
NQ = S // CHUNK              # 4 chunks
NT = HID // 128              # 16 hid tiles
SCALE = 1.0 / np.sqrt(np.float32(D))

F32 = mybir.dt.float32
F32R = mybir.dt.float32r
BF16 = mybir.dt.bfloat16
Alu = mybir.AluOpType
Act = mybir.ActivationFunctionType
BFNP = ml_dtypes.bfloat16

_CACHE = {}

# transpose-mode diag-matmuls write bf16 PSUM (2x DVE copies); falls back to
# regular fp32 matmuls when disabled
TMODE = os.environ.get("FA_TMODE", "1") == "1"


def _rope_tables():
    inv_freq = 1.0 / (THETA ** (np.arange(0, D, 2, dtype=np.float32) / D))
    pos = np.arange(S, dtype=np.float32)
    freqs = pos[:, None].astype(np.float32) * inv_freq[None, :]
    emb = np.concatenate([freqs, freqs], axis=-1)  # [S, D]
    cosT = np.cos(emb).astype(np.float32).T.copy()
    sinT = np.sin(emb).astype(np.float32).T.copy()
    return cosT, sinT  # [D, S]


def _classify_mask(attention_mask):
    """Per (qi, j) CHUNKxCHUNK block: 'zero' | 'neg' | 'tri' (canonical causal
    diagonal), merged across batches so the SPMD program is identical on all
    cores. Only pure-causal masks are supported by this kernel."""
    q = np.arange(CHUNK)
    tri_full = np.where(q[:, None] >= q[None, :], 0.0, NEG).astype(np.float32)
    kinds = {}
    for qi in range(NQ):
        for j in range(NQ):
            kind = None
            for b in range(B):
                blk = attention_mask[b, 0, qi * CHUNK:(qi + 1) * CHUNK,
                                     j * CHUNK:(j + 1) * CHUNK]
                if np.all(blk == 0.0):
                    k = "zero"
                elif np.all(blk <= -1e6):
                    k = "neg"
                elif np.array_equal(blk, tri_full):
                    k = "tri"
                else:
                    raise NotImplementedError("non-causal mask block")
                if kind is None:
                    kind = k
                elif kind != k:
                    raise NotImplementedError("mask differs across batches")
            kinds[(qi, j)] = kind
    plan = {}
    for qi in range(NQ):
        processed = []
        for j in range(NQ):
            k = kinds[(qi, j)]
            if k == "neg" and len(processed) > 0:
                continue  # identity step under the reference's fp32 exp underflow
            assert k != "neg" or len(processed) == 0
            if k == "neg":
                # leading fully-masked chunk: contributes T=0 rows; unsupported
                raise NotImplementedError("leading all-neg chunk")
            processed.append((j, k == "tri"))
        plan[qi] = processed
    return plan


def _mm(nc, out, lhsT, rhs, start, stop):
    nc.tensor.matmul(out, lhsT, rhs, start=start, stop=stop)


def _emit(tc, ap, plan):
    nc = tc.nc

    with ExitStack() as top:
        # ---------------- persistent tensors ----------------
        pers = top.enter_context(tc.tile_pool(name="pers", bufs=1))
        KT = pers.tile([128, KVL, S], BF16)            # rope'd k^T  [d, kv, s]
        V = pers.tile([128, S // 128, KVL * D], BF16)  # v natural [s_p, s_t, kv*d]
        xt_pool = top.enter_context(tc.tile_pool(name="xt", bufs=2))
        qt_pool = top.enter_context(tc.tile_pool(name="qtp", bufs=2))
        hsT_r = ap["hsT"].rearrange("(t p) s -> p t s", p=128)

        xts = {}

        def load_xt(sq):
            xt = xt_pool.tile([128, NT, CHUNK], BF16, tag="xt")
            ssl = slice(sq * CHUNK, (sq + 1) * CHUNK)
            for tq in range(4):
                nc.sync.dma_start(xt[:, tq * 4:(tq + 1) * 4, :],
                                  hsT_r[:, tq * 4:(tq + 1) * 4, ssl])
            xts[sq] = xt

        # startup DMAs ordered by first use: first-half weights + first x
        # chunk + rope tables first, everything else behind them
        wqk_sb = pers.tile([128, NT, (HL + KVL) * 128], BF16)
        wqk_r = ap["wqk"].rearrange("(t p) m -> p t m", p=128)
        ssl0 = slice(0, CHUNK)
        xt0 = xt_pool.tile([128, NT, CHUNK], BF16, tag="xt")
        xts[0] = xt0
        nc.sync.dma_start(wqk_sb[:, :2], wqk_r[:, :2])
        nc.sync.dma_start(xt0[:, :2, :], hsT_r[:, :2, ssl0])
        nc.sync.dma_start(wqk_sb[:, 2:4], wqk_r[:, 2:4])
        nc.sync.dma_start(xt0[:, 2:4, :], hsT_r[:, 2:4, ssl0])
        for tq in range(1, 4):
            nc.sync.dma_start(wqk_sb[:, tq * 4:(tq + 1) * 4],
                              wqk_r[:, tq * 4:(tq + 1) * 4])
            nc.sync.dma_start(xt0[:, tq * 4:(tq + 1) * 4, :],
                              hsT_r[:, tq * 4:(tq + 1) * 4, ssl0])
        cosT = pers.tile([128, S], BF16)
        sinT = pers.tile([128, S], BF16)
        nc.sync.dma_start(cosT[:, ssl0], ap["cosT"][:, ssl0])
        nc.sync.dma_start(sinT[:, ssl0], ap["sinT"][:, ssl0])
        R128 = pers.tile([128, 128], F32R)
        nc.sync.dma_start(R128, ap["rmat"])
        bqk = pers.tile([128, HL + KVL], F32)
        nc.sync.dma_start(bqk, ap["bqk"])
        wv_sb = pers.tile([128, NT, KVL * D], BF16)
        nc.sync.dma_start(wv_sb[:, :4], ap["wv"].rearrange("(t p) m -> p t m", p=128)[:, :4])
        nc.sync.dma_start(wv_sb[:, 4:], ap["wv"].rearrange("(t p) m -> p t m", p=128)[:, 4:])
        bv = pers.tile([1, KVL * D], F32R)
        nc.sync.dma_start(bv, ap["bv"])
        ones1 = pers.tile([1, 128], F32R)
        nc.sync.dma_start(ones1, ap["ones1"])
        I128b = pers.tile([128, 128], BF16)
        nc.sync.dma_start(I128b, ap["imatb"])
        triN = pers.tile([128, 128], BF16)
        nc.sync.dma_start(triN, ap["triN"])
        load_xt(1)
        for cq in range(1, NQ):
            cs = slice(cq * CHUNK, (cq + 1) * CHUNK)
            nc.sync.dma_start(cosT[:, cs], ap["cosT"][:, cs])
            nc.sync.dma_start(sinT[:, cs], ap["sinT"][:, cs])
        wo_sb = pers.tile([128, HL, HID], BF16)
        wo_r = ap["wo"].rearrange("(t p) m -> p t m", p=128)
        for mo in range(4):
            nc.sync.dma_start(wo_sb[:, :, mo * 512:(mo + 1) * 512],
                              wo_r[:, :, mo * 512:(mo + 1) * 512])

        # ---------------- pools (single scope; PSUM budget = 8 banks) ------
        raw_pool = top.enter_context(tc.tile_pool(name="raw", bufs=2))
        t_pool = top.enter_context(tc.tile_pool(name="ropetmp", bufs=2))
        ps_proj = top.enter_context(tc.tile_pool(name="psproj", bufs=3, space="PSUM"))
        ps_att = top.enter_context(tc.tile_pool(name="psatt", bufs=4, space="PSUM"))
        u_ps = top.enter_context(tc.tile_pool(name="ups", bufs=1, space="PSUM"))

        p_pool = top.enter_context(tc.tile_pool(name="pstore", bufs=2))
        pt_pool = top.enter_context(tc.tile_pool(name="ptrans", bufs=2))
        ch_pool = top.enter_context(tc.tile_pool(name="chain", bufs=2))
        o2_pool = top.enter_context(tc.tile_pool(name="uout", bufs=2))
        o_pool = top.enter_context(tc.tile_pool(name="osb", bufs=4))

        QTs = {}
        pend_rope = []

        def rope_tail(sq, m, raw):
            ssl = slice(sq * CHUNK, (sq + 1) * CHUNK)
            pr = ps_proj.tile([128, CHUNK], F32, tag="pp")
            _mm(nc, pr, R128, raw, start=True, stop=True)
            est["pe"] += CHUNK * PEC
            t1 = t_pool.tile([128, CHUNK], F32, tag="t1")
            nc.gpsimd.tensor_mul(t1, raw.bitcast(F32), cosT[:, ssl])
            t2 = t_pool.tile([128, CHUNK], F32, tag="t2")
            nc.vector.tensor_mul(t2, pr, sinT[:, ssl])
            est["dve"] = max(est["dve"], est["pe"] + 173.0) + (CHUNK + 120) * 1.04
            dest = QTs[sq][:, m, :] if m < HL else KT[:, m - HL, ssl]
            nc.gpsimd.tensor_add(dest, t1, t2)
            est["pool"] = max(est["pool"], est["dve"]) + 2 * (CHUNK * 0.833 / 0.42 + 95)

        def flush_rope(upto=0):
            while len(pend_rope) > upto:
                rope_tail(*pend_rope.pop(0))

        def proj_m_unit(sq, m):
            # one head-dim block of the q/k projection, as a generator that
            # yields every 4 K-tiles so filler drains stay fine-grained;
            # rope tail of the PREVIOUS m runs behind this chain so the PE
            # never waits on the DVE bias add
            xt = xts[sq]
            if m == 0:
                QTs[sq] = qt_pool.tile([128, HL, CHUNK], BF16, tag="qt",
                                       name=f"qt{sq}")
            ps = ps_proj.tile([128, CHUNK], F32, tag="pp")
            for t in range(NT):
                _mm(nc, ps, wqk_sb[:, t, m * 128:(m + 1) * 128], xt[:, t],
                    start=(t == 0), stop=(t == NT - 1))
                est["pe"] += CHUNK * PEC
                if t % 4 == 3 and t < NT - 1:
                    yield
            raw = raw_pool.tile([128, CHUNK], F32R)
            nc.vector.tensor_scalar_add(raw, ps, bqk[:, m:m + 1])
            est["dve"] = max(est["dve"], est["pe"] + 173.0) + (CHUNK + 120) * 1.04
            pend_rope.append((sq, m, raw))
            if len(pend_rope) > 1:
                rope_tail(*pend_rope.pop(0))
            proj_pending[sq] -= 1

        def proj_v_unit(sq, ss):
            # v projection (natural layout), bias via K=1 matmul; runs on the
            # still-loaded x chunk behind the qk projection
            flush_rope()
            xt = xts[sq]
            pv = ps_proj.tile([128, CHUNK], F32, tag="pp")
            for t in range(NT):
                _mm(nc, pv[:, :KVL * D], xt[:, t, ss * 128:(ss + 1) * 128], wv_sb[:, t],
                    start=(t == 0), stop=False)
                est["pe"] += KVL * D * PEC
                if t % 6 == 5 and t < NT - 1:
                    yield
            _mm(nc, pv[:, :KVL * D], ones1, bv, start=False, stop=True)
            est["pe"] += KVL * D * PEC
            nc.vector.tensor_copy(V[:, sq * 4 + ss, :], pv[:, :KVL * D])
            est["dve"] = max(est["dve"], est["pe"] + 173.0) + (KVL * D + 120) * 1.04
            proj_pending[sq] -= 1

        # ---- emission-time cost estimator: models each engine's in-order
        # stream so fillers are drained exactly when the PE stream would
        # stall, and psum->sbuf copies go to whichever of Act/DVE finishes
        # first. Pure scheduling heuristic - semantics are unaffected.
        est = {"pe": 0.0, "act": 0.0, "dve": 0.0, "pool": 0.0, "dma": 0.0}
        exp_hist = []   # completion times of pass1 exps (ps_att ring)
        PEC = 1.0 / 2.4

        def pe_ready(need, drain):
            while est["pe"] < need - 50.0 and fillq_has():
                drain(1)
            est["pe"] = max(est["pe"], need)

        # ---- pass1 for one (qi, h): scores once, exp into resident P,
        # raw sums via Act accumulate, row maxes via DVE; exp-domain chain.
        # PV steps of the PREVIOUS head weave between score tiles (pump)
        # so the in-order PE stream stays packed at the Act engine's pace.
        def pump(prev_pv, drain):
            while est["act"] - est["pe"] > 100.0:
                if (prev_pv is not None and prev_pv["idx"] < len(prev_pv["steps"])
                        and prev_pv["ready"] <= est["pe"] + 100.0):
                    emit_pv(prev_pv)
                elif fillq:
                    drain(1)
                else:
                    break

        def pass1_unit(qi, h, prev_pv, drain):
            chunks = plan[qi]
            nj = len(chunks)
            QT = QTs[qi]
            P = p_pool.tile([128, 4, nj, CHUNK], BF16, tag="P",
                            name=f"P{qi}_{h}")
            mxe = ch_pool.tile([128, nj, 4], F32, tag="mxe", name=f"mxe{qi}_{h}")
            sraw = ch_pool.tile([128, nj, 4], F32, tag="sraw", name=f"sr{qi}_{h}")
            for t, (j, diag) in enumerate(chunks):
                k0 = j * CHUNK
                for sub in range(4):
                    q0 = sub * 128
                    w = (sub + 1) * 128 if diag else CHUNK
                    # ps_att ring: this score reuses the bank freed by the
                    # exp 3 tiles back
                    if len(exp_hist) >= 3:
                        pe_ready(exp_hist[-3], drain)
                    ps = ps_att.tile([128, CHUNK], F32, tag="ps")
                    _mm(nc, ps[:, :w], QT[:, h, q0:q0 + 128],
                        KT[:, h // 2, k0:k0 + w],
                        start=True, stop=not diag)
                    est["pe"] += w * PEC
                    if diag:
                        _mm(nc, ps[:, w - 128:w], I128b, triN,
                            start=False, stop=True)
                        est["pe"] += 128 * PEC
                    nc.scalar.activation(
                        P[:, sub, t, :w], ps[:, :w], Act.Exp,
                        accum_out=sraw[:, t, sub:sub + 1])
                    e_done = max(est["act"], est["pe"] + 173.0) \
                        + (w + 222) * 0.833 + 187
                    est["act"] = e_done
                    exp_hist.append(e_done)
                    nc.vector.tensor_reduce(
                        mxe[:, t, sub:sub + 1], P[:, sub, t, :w],
                        axis=mybir.AxisListType.X, op=Alu.max)
                    est["dve"] = max(est["dve"], e_done) + (w + 58) * 1.04
                    pump(prev_pv, drain)
            return {"qi": qi, "h": h, "nj": nj, "chunks": chunks,
                    "P": P, "mxe": mxe, "sraw": sraw}

        def chain_unit(st):
            # exp-domain chain on DVE:
            #   M_t = running max of mxe  (M_{-1} = 0)
            #   d_t = (M_{t-1} + S_t) / M_t
            #   c_t = 1 / (M_fin * prod_{l>=t} d_l * d_last^flag)
            qi, h, nj = st["qi"], st["h"], st["nj"]
            mxe, sraw = st["mxe"], st["sraw"]
            Mrun = ch_pool.tile([128, nj + 1, 4], F32, tag="Mrun",
                                name=f"Mr{qi}_{h}")
            nc.vector.memset(Mrun[:, 0, :], 0.0)
            for t in range(nj):
                nc.vector.tensor_tensor(Mrun[:, t + 1, :], Mrun[:, t, :],
                                        mxe[:, t, :], Alu.max)
            num = ch_pool.tile([128, nj, 4], F32, tag="num", name=f"nm{qi}_{h}")
            nc.vector.tensor_add(num, Mrun[:, :nj, :], sraw)
            rM = ch_pool.tile([128, nj, 4], F32, tag="rM", name=f"rM{qi}_{h}")
            nc.vector.reciprocal(rM, Mrun[:, 1:, :])
            dq = ch_pool.tile([128, nj, 4], F32, tag="dq", name=f"dq{qi}_{h}")
            nc.vector.tensor_mul(dq, num, rM)
            if any(j == NQ - 1 for (j, _) in st["chunks"]):
                nc.vector.tensor_mul(dq[:, nj - 1, :], dq[:, nj - 1, :],
                                     dq[:, nj - 1, :])
            # suffix products G_t = M_fin * prod_{l>=t} d_l
            G = ch_pool.tile([128, nj + 1, 4], F32, tag="G", name=f"G{qi}_{h}")
            nc.vector.tensor_copy(G[:, nj, :], Mrun[:, nj, :])
            for t in range(nj - 1, -1, -1):
                nc.vector.tensor_mul(G[:, t, :], dq[:, t, :], G[:, t + 1, :])
            cc = ch_pool.tile([128, nj, 4], F32, tag="cc", name=f"cc{qi}_{h}")
            nc.vector.reciprocal(cc, G[:, :nj, :])
            est["dve"] += (3 * nj + 7) * 77.0
            # prescale P in place by c_t (per-partition scalar in [q,k]
            # layout - the ONLY layout where the scale axis is the partition
            # axis), then transpose each q-sub strip on the DMA XBAR:
            # PT[kp, t, slab, sub, q] = c_t[q]*P[q, sub, t, slab*128+kp]
            P = st["P"]
            PT = pt_pool.tile([128, nj, 4, 4, 128], BF16, tag="PT",
                              name=f"PT{qi}_{h}")
            dma_done = est["dve"] + 1300.0
            for sub in range(4):
                for t, (j, diag) in enumerate(st["chunks"]):
                    w = (sub + 1) * 128 if diag else CHUNK
                    nc.vector.tensor_scalar_mul(P[:, sub, t, :w],
                                                P[:, sub, t, :w],
                                                cc[:, t, sub:sub + 1])
                    est["dve"] += (w * 0.26 + 58) * 1.04
                nc.sync.dma_start_transpose(PT[:, :, :, sub, :],
                                            P[:, sub, :, :])
                dma_done = max(est["dma"], est["dve"] + 1300.0) + nj * 64 * 14.0
                est["dma"] = dma_done
            st["PT"] = PT
            st["pt_done"] = dma_done

        def psum_copy(dst, src, w):
            # route the psum->sbuf copy to whichever engine finishes first
            # (start gated on the producing matmul via est["pe"])
            src_done = est["pe"] + 173.0
            a_fin = max(est["act"], src_done) + (w + 222) * 0.833
            d_fin = max(est["dve"], src_done) + (w + 120) * 1.04
            if a_fin <= d_fin:
                nc.scalar.activation(dst, src, Act.Copy)
                est["act"] = a_fin
                return a_fin
            nc.vector.tensor_copy(dst, src)
            est["dve"] = d_fin
            return d_fin

        # ---- PV micro-units: u[d, q] = sum_{t,kc} V_slab^T @ PT_slab; the
        # transposed+scaled P strips arrive from the DMA XBAR. Steps are
        # emitted one at a time by pump()/flush_pv().
        def make_pv(st):
            st["steps"] = [(t, j, diag, kc)
                           for t, (j, diag) in enumerate(st["chunks"])
                           for kc in range(4)]
            st["idx"] = 0
            st["ready"] = st["pt_done"]
            st["up"] = u_ps.tile([128, CHUNK], F32, tag="up",
                                 name=f"up{st['qi']}_{st['h']}")
            return st

        def emit_pv(pv):
            i = pv["idx"]
            t, j, diag, kc = pv["steps"][i]
            h = pv["h"]
            off = kc * 128 if diag else 0
            PT = pv["PT"]
            rhs = PT[:, t, kc, kc:, :] if diag else PT[:, t, kc, :, :]
            _mm(nc, pv["up"][:, off:],
                V[:, j * 4 + kc, (h // 2) * D:(h // 2 + 1) * D],
                rhs, start=(i == 0), stop=(i == len(pv["steps"]) - 1))
            est["pe"] += (CHUNK - off) * PEC
            pv["idx"] = i + 1

        def flush_pv(pv, drain):
            if pv is None:
                return None
            if pv["idx"] < len(pv["steps"]):
                pe_ready(pv["ready"], drain)
                while pv["idx"] < len(pv["steps"]):
                    emit_pv(pv)
            h, qi = pv["h"], pv["qi"]
            ub = o2_pool.tile([128, CHUNK], BF16, tag=f"ub{h}",
                              name=f"ub{h}_{qi}")
            nc.vector.tensor_copy(ub, pv["up"])
            est["dve"] = max(est["dve"], est["pe"] + 173.0) + (CHUNK + 120) * 1.04
            return ub

        def wo_unit(qi, ubs, mo):
            # one output-projection tile
            qsl = slice(qi * CHUNK, (qi + 1) * CHUNK)
            po = ps_proj.tile([128, CHUNK], F32, tag="pp")
            for t in range(HL):
                _mm(nc, po, wo_sb[:, t, mo * 128:(mo + 1) * 128], ubs[t],
                    start=(t == 0), stop=(t == HL - 1))
            est["pe"] += HL * CHUNK * PEC
            ob = o_pool.tile([128, CHUNK], BF16)
            psum_copy(ob, po, CHUNK)
            nc.sync.dma_start(ap["outT"][mo * 128:(mo + 1) * 128, qsl], ob)

        # ---------------- schedule ----------------
        # stage qi: the Act/DVE-heavy attention units (pass1 exp/max ->
        # chain -> pass2 transpose+PV) run with a FIFO queue of PE-heavy
        # filler units (next chunk's qk/v projections, previous chunk's
        # output projection) drained into the in-order PE stream whenever
        # the attention stream would stall it.
        ub_store = {}
        fillq = []
        proj_pending = [0] * (NQ + 1)

        def fillq_has():
            return len(fillq) > 0

        def drain(n):
            done = 0
            while done < n and fillq:
                try:
                    next(fillq[0])
                    done += 1
                except StopIteration:
                    fillq.pop(0)

        def wo_gen(qi, ubs, mo):
            wo_unit(qi, ubs, mo)
            yield

        def make_proj_units(sq):
            units = [proj_m_unit(sq, m) for m in range(HL + KVL)]
            units += [proj_v_unit(sq, ss) for ss in range(4)]
            proj_pending[sq] += len(units)
            return units

        fillq.extend(make_proj_units(0))
        while proj_pending[0] > 0 and fillq:
            drain(1)
        for qi in range(NQ):
            if qi + 1 < NQ:
                if qi + 1 not in xts:
                    load_xt(qi + 1)
                fillq.extend(make_proj_units(qi + 1))
            ubs = []
            prev_pv = None
            for h in range(HL):
                st = pass1_unit(qi, h, prev_pv, drain)
                chain_unit(st)
                ubs.append(flush_pv(prev_pv, drain))
                prev_pv = make_pv(st)
            ubs.append(flush_pv(prev_pv, drain))
            ubs = [u for u in ubs if u is not None]
            ub_store[qi] = ubs
            fillq.extend(
                wo_gen(qi, ubs, mo) for mo in range(HID // 128))
            while proj_pending[qi + 1] > 0 and fillq:
                drain(1)
        while fillq:
            drain(1)


def _build_program(plan):
    nc = bacc.Bacc("TRN2", target_bir_lowering=False, debug=False,
                   enable_asserts=False, num_devices=NCORES)
    ap = {}
    ap["hsT"] = nc.dram_tensor("hsT", [HID, S], BF16, kind="ExternalInput").ap()
    ap["wqk"] = nc.dram_tensor("wqk", [HID, (HL + KVL) * D], BF16, kind="ExternalInput").ap()
    ap["wv"] = nc.dram_tensor("wv", [HID, KVL * D], BF16, kind="ExternalInput").ap()
    ap["wo"] = nc.dram_tensor("wo", [HL * D, HID], BF16, kind="ExternalInput").ap()
    ap["bqk"] = nc.dram_tensor("bqk", [D, HL + KVL], F32, kind="ExternalInput").ap()
    ap["bv"] = nc.dram_tensor("bv", [1, KVL * D], F32R, kind="ExternalInput").ap()
    ap["cosT"] = nc.dram_tensor("cosT", [D, S], BF16, kind="ExternalInput").ap()
    ap["sinT"] = nc.dram_tensor("sinT", [D, S], BF16, kind="ExternalInput").ap()
    ap["rmat"] = nc.dram_tensor("rmat", [D, D], F32R, kind="ExternalInput").ap()
    ap["imatb"] = nc.dram_tensor("imatb", [128, 128], BF16, kind="ExternalInput").ap()
    ap["triN"] = nc.dram_tensor("triN", [128, 128], BF16, kind="ExternalInput").ap()
    ap["ones1"] = nc.dram_tensor("ones1", [1, 128], F32R, kind="ExternalInput").ap()
    ap["outT"] = nc.dram_tensor("outT", [HID, S], BF16, kind="ExternalOutput").ap()

    with tile.TileContext(nc) as tc:
        _emit(tc, ap, plan)
    nc.compile()
    return nc


def _host_inputs(inputs):
    hs = np.asarray(inputs["hidden_states"], dtype=np.float32)
    Wq = np.asarray(inputs["Wq"], dtype=np.float32)
    bq = np.asarray(inputs["bq"], dtype=np.float32)
    Wk = np.asarray(inputs["Wk"], dtype=np.float32)
    bk = np.asarray(inputs["bk"], dtype=np.float32)
    Wv = np.asarray(inputs["Wv"], dtype=np.float32)
    bv_ = np.asarray(inputs["bv"], dtype=np.float32)
    Wo = np.asarray(inputs["Wo"], dtype=np.float32)

    cosT, sinT = _rope_tables()
    R = np.zeros((D, D), dtype=np.float32)
    R[64 + np.arange(64), np.arange(64)] = -1.0   # out[d'<64] = -q[d'+64]
    R[np.arange(64), 64 + np.arange(64)] = 1.0    # out[d'>=64] = q[d'-64]
    I = np.eye(128, dtype=np.float32)
    q = np.arange(128)
    triN = np.where(q[:, None] >= q[None, :], 0.0, NEG).astype(BFNP)

    Wq4 = (Wq * SCALE).reshape(HID, H, D)
    bq4 = (bq * SCALE).reshape(H, D)
    Wk4 = Wk.reshape(HID, HKV, D)
    bk4 = bk.reshape(HKV, D)
    Wv4 = Wv.reshape(HID, HKV, D)
    bv4 = bv_.reshape(HKV, D)
    Wo4 = Wo.reshape(H, D, HID)

    in_maps = []
    for c in range(NCORES):
        b, hg = divmod(c, NCORES // B)
        qh = slice(hg * HL, (hg + 1) * HL)
        kvh = slice(hg * KVL, (hg + 1) * KVL)
        wqk = np.concatenate([
            Wq4[:, qh].reshape(HID, HL * D),
            Wk4[:, kvh].reshape(HID, KVL * D)], axis=1)
        bqk = np.concatenate([bq4[qh], bk4[kvh]], axis=0).T  # [D, HL+KVL]
        in_maps.append({
            "hsT": hs[b].T.astype(BFNP),
            "wqk": wqk.astype(BFNP),
            "wv": Wv4[:, kvh].reshape(HID, KVL * D).astype(BFNP),
            "wo": Wo4[qh].reshape(HL * D, HID).astype(BFNP),
            "bqk": np.ascontiguousarray(bqk),
            "bv": bv4[kvh].reshape(1, KVL * D).copy(),
            "cosT": cosT.astype(BFNP),
            "sinT": sinT.astype(BFNP),
            "rmat": R,
            "imatb": I.astype(BFNP),
            "triN": triN,
            "ones1": np.ones((1, 128), dtype=np.float32),
        })
    return in_maps


def get_program(inputs):
    am = np.asarray(inputs["attention_mask"], dtype=np.float32)
    plan = _classify_mask(am)
    key = str(plan)
    if key not in _CACHE:
        _CACHE[key] = _build_program(plan)
    return _CACHE[key], plan, None


def run(inputs, **spmd_kwargs):
    nc, plan, _ = get_program(inputs)
    in_maps = _host_inputs(inputs)
    res = run_bass_kernel_spmd(nc, in_maps, core_ids=list(range(NCORES)),
                               **spmd_kwargs)
    bo = np.asarray(inputs["bo"], dtype=np.float32)
    out = np.empty((B, S, HID), dtype=np.float32)
    gpb = NCORES // B
    for b in range(B):
        acc = np.zeros((HID, S), dtype=np.float32)
        for c in range(b * gpb, (b + 1) * gpb):
            acc += np.asarray(res.results[c]["outT"]).astype(np.float32)
        out[b] = acc.T + bo
    return out, res


def kernel(**inputs) -> np.ndarray:
    out, _ = run(inputs)
    return out
